# revision 4
# baseline (speedup 1.0000x reference)
"""Distributed causal self-attention kernel for 8 Trainium2 NeuronCores.

Problem: B=4, T=2048, D=1024, H=16 heads (head_dim 64), fp32 I/O.
    qkv = x @ w_qkv + b_qkv; causal softmax attention (scale 1/sqrt(D));
    out = attn_out @ w_out + b_out.

Sharding: 8 cores = 4 batches x 2 head-halves. Each core computes one
batch's attention for 8 heads plus its partial output projection
(rows of w_out for its head dims); host sums the two partials per batch.

Device-side layout (all matmul operands fp16, PSUM accumulation fp32):
  - host pre-transposes x -> xT [D, T] and pre-slices/casts weights
  - q^T,k^T [1024, T] = wqk.T @ xT  (PE, stationary = wqk tiles)
  - v [T, 512(+ones cols)] natural   (PE, stationary = xT tiles)
  - S^T[k,q] = K @ Q^T per head, causal lower blocks only; head PAIRS run
    concurrently on the PE via row-packing (K=64 each, base partitions 0/64)
  - P^T = exp(S^T * 1/32) on ScalarE straight out of PSUM (trimmed to the
    valid causal range); diagonal 128x128 subblocks masked on VectorE
  - O^T[65,q] = [V|1].T @ P^T accumulated in PSUM; row 64 = softmax denom
  - normalize with reciprocal_approx_fast + column-broadcast multiply
  - y_partial [T, D] = O^T.T @ wo_rows  (PE), DMA out fp32
"""

import numpy as np

B, T, D, H = 4, 2048, 1024, 16
HD = 64          # head dim
HPC = 8          # heads per core
DH = HPC * HD    # 512: head dims per core
NCORES = 8
SCALE = 1.0 / 32.0  # 1/sqrt(D)

_cache = {}


def _build_nc():
    import concourse.bacc as bacc
    import concourse.mybir as mybir
    import concourse.tile as tile

    f16 = mybir.dt.float16
    f32 = mybir.dt.float32
    Exp = mybir.ActivationFunctionType.Exp

    nc = bacc.Bacc()

    xT = nc.dram_tensor("xT", [D, T], f16, kind="ExternalInput")
    wqk = nc.dram_tensor("wqk", [D, 2 * DH], f16, kind="ExternalInput")
    wv = nc.dram_tensor("wv", [D, DH], f16, kind="ExternalInput")
    wo = nc.dram_tensor("wo", [DH, D], f16, kind="ExternalInput")
    bqk = nc.dram_tensor("bqk", [128, 8], f32, kind="ExternalInput")
    bv = nc.dram_tensor("bv", [128, DH], f32, kind="ExternalInput")
    mask = nc.dram_tensor("mask", [128, 128], f16, kind="ExternalInput")
    y = nc.dram_tensor("y", [T, D], f32, kind="ExternalOutput")

    xT_t = xT.rearrange("(k p) t -> k p t", p=128)     # 8 x [128, 2048]
    wqk_t = wqk.rearrange("(k p) n -> k p n", p=128)   # 8 x [128, 1024]
    wv_t = wv.rearrange("(k p) n -> k p n", p=128)     # 8 x [128, 512]
    wo_t = wo.rearrange("(k p) n -> k p n", p=128)     # 4 x [128, 1024]

    with tile.TileContext(nc) as tc:
        with (
            tc.tile_pool(name="consts", bufs=1) as cp,
            tc.tile_pool(name="ptp", bufs=4) as ptp,
            tc.tile_pool(name="recp", bufs=4) as recp,
            tc.tile_pool(name="ysb", bufs=3) as ysb,
            tc.tile_pool(name="psum", bufs=4, space="PSUM") as psp,
        ):
            xt_sb = [cp.tile([128, T], f16, name=f"xt{k}", tag=f"xt{k}") for k in range(8)]
            wqk_sb = [cp.tile([128, 2 * DH], f16, name=f"wqk{k}", tag=f"wqk{k}") for k in range(8)]
            wv_sb = [cp.tile([128, DH], f16, name=f"wv{k}", tag=f"wv{k}") for k in range(8)]
            wo_sb = [cp.tile([128, D], f16, name=f"wo{k}", tag=f"wo{k}") for k in range(4)]
            bqk_sb = cp.tile([128, 8], f32, name="bqk_sb", tag="bqk_sb")
            bv_sb = cp.tile([128, DH], f32, name="bv_sb", tag="bv_sb")
            mask_sb = cp.tile([128, 128], f16, name="mask_sb", tag="mask_sb")
            # q^T on tiles 0-3 (head pairs), k^T on tiles 4-7
            qk_sb = [cp.tile([128, T], f16, name=f"qk{r}", tag=f"qk{r}") for r in range(8)]
            # v natural, 65 cols per head (64 dims + ones)
            v_sb = [cp.tile([128, HPC * 65], f16, name=f"v{t}", tag=f"v{t}") for t in range(16)]
            oT_sb = [cp.tile([128, T], f16, name=f"oT{i}", tag=f"oT{i}") for i in range(4)]

            for k in range(8):
                nc.sync.dma_start(xt_sb[k][:], xT_t[k])
                nc.sync.dma_start(wqk_sb[k][:], wqk_t[k])
            for k in range(8):
                nc.sync.dma_start(wv_sb[k][:], wv_t[k])
            nc.sync.dma_start(bqk_sb[:], bqk[:])
            nc.sync.dma_start(bv_sb[:], bv[:])
            nc.sync.dma_start(mask_sb[:], mask[:])
            for k in range(4):
                nc.sync.dma_start(wo_sb[k][:], wo_t[k])

            # ones columns of v tiles (col 64 of each 65-col head group)
            for t in range(16):
                vv = v_sb[t][:].rearrange("p (h c) -> p h c", c=65)
                nc.gpsimd.memset(vv[:, :, 64:65], 1.0)

            # ---- Phase 1a: q^T / k^T = wqk.T @ xT + bias ----
            for r in range(8):
                pts = []
                for c in range(4):
                    pt = psp.tile([128, 512], f32, name=f"qkp{r}_{c}", tag="mm")
                    pts.append(pt)
                for k in range(8):
                    for c in range(4):
                        nc.tensor.matmul(
                            pts[c][:],
                            wqk_sb[k][:, r * 128:(r + 1) * 128],
                            xt_sb[k][:, c * 512:(c + 1) * 512],
                            start=(k == 0), stop=(k == 7),
                        )
                for c in range(4):
                    nc.vector.tensor_scalar_add(
                        qk_sb[r][:, c * 512:(c + 1) * 512], pts[c][:],
                        bqk_sb[:, r:r + 1],
                    )

            # ---- Phase 1b: v natural = xT.T @ wv + bias ----
            bv_b = bv_sb[:].rearrange("p (h c) -> p h c", c=64)
            for t in range(16):
                pv = psp.tile([128, 512], f32, name=f"vp{t}", tag="mm")
                for k in range(8):
                    nc.tensor.matmul(
                        pv[:],
                        xt_sb[k][:, t * 128:(t + 1) * 128],
                        wv_sb[k][:],
                        start=(k == 0), stop=(k == 7),
                    )
                vdst = v_sb[t][:].rearrange("p (h c) -> p h c", c=65)[:, :, 0:64]
                vsrc = pv[:].rearrange("p (h c) -> p h c", c=64)
                nc.vector.tensor_add(vdst, vsrc, bv_b)

            # ---- Phase 2: attention per head pair i ----
            for i in range(4):
                qa = qk_sb[i][0:64, :]
                qb = qk_sb[i][64:128, :]
                ka = qk_sb[4 + i][0:64, :]
                kb = qk_sb[4 + i][64:128, :]
                for g in range(4):
                    oa = psp.tile([65, 512], f32, name=f"oa{i}_{g}", tag="pv")
                    ob = psp.tile([65, 512], f32, name=f"ob{i}_{g}", tag="pv")
                    nj = 4 * g + 4
                    for j in range(nj):
                        o = max(0, 128 * j - 512 * g)
                        w = 512 - o
                        q0 = 512 * g + o
                        sa = psp.tile([128, 512], f32, name=f"sa{i}_{g}_{j}", tag="mm")
                        sb_ = psp.tile([128, 512], f32, name=f"sb{i}_{g}_{j}", tag="mm")
                        nc.tensor.matmul(
                            sa[:, o:512], ka[:, j * 128:(j + 1) * 128],
                            qa[:, q0:q0 + w], start=True, stop=True)
                        nc.tensor.matmul(
                            sb_[:, o:512], kb[:, j * 128:(j + 1) * 128],
                            qb[:, q0:q0 + w], start=True, stop=True)
                        pa = ptp.tile([128, 512], f16, name=f"pa{i}_{g}_{j}", tag="pa")
                        pb = ptp.tile([128, 512], f16, name=f"pb{i}_{g}_{j}", tag="pb")
                        nc.scalar.activation(pa[:, o:512], sa[:, o:512], Exp, scale=SCALE)
                        nc.scalar.activation(pb[:, o:512], sb_[:, o:512], Exp, scale=SCALE)
                        if j >= 4 * g:  # partial block: mask diag subblock
                            nc.vector.tensor_mul(
                                pa[:, o:o + 128], pa[:, o:o + 128], mask_sb[:])
                            nc.vector.tensor_mul(
                                pb[:, o:o + 128], pb[:, o:o + 128], mask_sb[:])
                        va = v_sb[j][:, (2 * i) * 65:(2 * i) * 65 + 65]
                        vb = v_sb[j][:, (2 * i + 1) * 65:(2 * i + 1) * 65 + 65]
                        nc.tensor.matmul(
                            oa[:, o:512], va, pa[:, o:512],
                            start=(j == 0), stop=(j == nj - 1))
                        nc.tensor.matmul(
                            ob[:, o:512], vb, pb[:, o:512],
                            start=(j == 0), stop=(j == nj - 1))
                    sa_in = recp.tile([1, 512], f32, name=f"sain{i}_{g}", tag="sain")
                    sb_in = recp.tile([1, 512], f32, name=f"sbin{i}_{g}", tag="sbin")
                    ra = recp.tile([1, 512], f32, name=f"ra{i}_{g}", tag="ra")
                    rb = recp.tile([1, 512], f32, name=f"rb{i}_{g}", tag="rb")
                    rab = recp.tile([64, 512], f32, name=f"rab{i}_{g}", tag="rab")
                    rbb = recp.tile([64, 512], f32, name=f"rbb{i}_{g}", tag="rbb")
                    nc.vector.tensor_copy(sa_in[:], oa[64:65, :])
                    nc.vector.tensor_copy(sb_in[:], ob[64:65, :])
                    nc.vector.reciprocal_approx_fast(ra[:], sa_in[:])
                    nc.vector.reciprocal_approx_fast(rb[:], sb_in[:])
                    nc.gpsimd.partition_broadcast(rab[:], ra[:])
                    nc.gpsimd.partition_broadcast(rbb[:], rb[:])
                    g0 = 512 * g
                    nc.vector.tensor_mul(
                        oT_sb[i][0:64, g0:g0 + 512], oa[0:64, :], rab[:])
                    nc.vector.tensor_mul(
                        oT_sb[i][64:128, g0:g0 + 512], ob[0:64, :], rbb[:])

            # ---- Phase 3: y = O^T.T @ wo ----
            for t in range(16):
                ph = []
                for h2 in range(2):
                    p = psp.tile([128, 512], f32, name=f"yp{t}_{h2}", tag="mm")
                    ph.append(p)
                for k in range(4):
                    for h2 in range(2):
                        nc.tensor.matmul(
                            ph[h2][:],
                            oT_sb[k][:, t * 128:(t + 1) * 128],
                            wo_sb[k][:, h2 * 512:(h2 + 1) * 512],
                            start=(k == 0), stop=(k == 3),
                        )
                yt = ysb.tile([128, D], f32, name=f"yt{t}", tag="yt")
                nc.vector.tensor_copy(yt[:, 0:512], ph[0][:])
                nc.vector.tensor_copy(yt[:, 512:1024], ph[1][:])
                nc.sync.dma_start(y[t * 128:(t + 1) * 128, :], yt[:])

    nc.compile()
    return nc


def _prep_inputs(x, w_qkv, b_qkv, w_out):
    mask_np = np.triu(np.ones((128, 128), np.float16))
    in_maps = []
    for c in range(NCORES):
        b, hh = divmod(c, 2)
        h0 = hh * HPC * HD  # 0 or 512: offset into each of q/k/v col sections
        wq = w_qkv[:, h0:h0 + DH]
        wk = w_qkv[:, D + h0:D + h0 + DH]
        wv_ = w_qkv[:, 2 * D + h0:2 * D + h0 + DH]
        bq = b_qkv[h0:h0 + DH]
        bk = b_qkv[D + h0:D + h0 + DH]
        bv_ = b_qkv[2 * D + h0:2 * D + h0 + DH]
        in_maps.append({
            "xT": np.ascontiguousarray(x[b].T).astype(np.float16),
            "wqk": np.concatenate([wq, wk], axis=1).astype(np.float16),
            "wv": np.ascontiguousarray(wv_).astype(np.float16),
            "wo": np.ascontiguousarray(w_out[h0:h0 + DH, :]).astype(np.float16),
            "bqk": np.ascontiguousarray(
                np.concatenate([bq, bk]).reshape(8, 128).T).astype(np.float32),
            "bv": np.broadcast_to(bv_.astype(np.float32), (128, DH)).copy(),
            "mask": mask_np,
        })
    return in_maps


def kernel(x, w_qkv, b_qkv, w_out, b_out, _trace=False):
    from concourse.bass_utils import run_bass_kernel_spmd

    x = np.asarray(x, dtype=np.float32)
    w_qkv = np.asarray(w_qkv, dtype=np.float32)
    b_qkv = np.asarray(b_qkv, dtype=np.float32)
    w_out = np.asarray(w_out, dtype=np.float32)
    b_out = np.asarray(b_out, dtype=np.float32)

    if "nc" not in _cache:
        _cache["nc"] = _build_nc()
    nc = _cache["nc"]

    in_maps = _prep_inputs(x, w_qkv, b_qkv, w_out)
    res = run_bass_kernel_spmd(
        nc, in_maps, core_ids=list(range(NCORES)), trace=_trace)
    _cache["last_result"] = res

    out = np.empty((B, T, D), dtype=np.float32)
    for b in range(B):
        out[b] = res.results[2 * b]["y"] + res.results[2 * b + 1]["y"]
    out += b_out[None, None, :].astype(np.float32)
    return out
